# revision 1
# baseline (speedup 1.0000x reference)
"""Trainium2 Bass kernel for nn_Attention_18726057410699 (gnn_message_passing).

Math (per sample b):
  y        = local_feats[b] @ W_apair                       # [192, 256]
  binv     = binary_feats[b] @ W_bin + b_bin                # [128,128,256]
  z[i,j,k] = y[i,k] + y[j,k] + (binv[i,j,k] if i<128 and j<128 else 0)
  s[i,j]   = sigmoid( sum_k relu(z[i,j,k]) * w_att[k] + b_att )
  out[i,h] = sum_j s[i,j] * local_feats[b][j,h]

Sharding: data-parallel over batch B=8 -> 8 cores, one sample each.
Host prep is layout-only: per-sample transpose of binary_feats to put the
contraction channel on SBUF partitions, plus tiny weight reshapes.
"""

import numpy as np

B, N, H, L, C = 8, 192, 256, 128, 112
NIJ = L * L  # 16384
IB = 4  # i-rows per burst

_CACHE = {}


ENGINE_SEM = {
    "EngineType.PE": "PE_",
    "EngineType.DVE": "DVE_",
    "EngineType.Activation": "Activation_",
    "EngineType.Pool": "Pool_",
    "EngineType.SP": "SP_",
}


def _fix_sync_waits(nc):
    """walrus in this toolchain accepts at most ONE sync-wait per compute
    instruction.  Tile emits several.  Two safe rewrites:
      1. drop self waits (instruction waiting on its own engine/queue sem --
         always satisfied by in-order execution of per-proc sems);
      2. push overflow waits onto earlier same-engine instructions (waiting
         earlier on the same in-order engine is strictly more conservative).
    """
    import dataclasses
    from collections import defaultdict

    il = [i for i in nc.all_instructions()]
    streams = defaultdict(list)
    for inst in il:
        si = getattr(inst, "sync_info", None)
        if si is None:
            continue
        upd = {u.ant_name for u in si.on_update}
        eng = str(getattr(inst, "engine", None))
        self_pfx = ENGINE_SEM.get(eng)
        keep = {}
        for w in si.on_wait:
            if w.ant_name in upd:
                continue  # self queue/engine sem
            if self_pfx and w.ant_name.startswith(self_pfx):
                continue  # own engine sem
            k = w.ant_name
            if k not in keep or keep[k].wait_value < w.wait_value:
                keep[k] = w
        new = list(keep.values())
        if len(new) != len(si.on_wait):
            inst.sync_info = dataclasses.replace(si, on_wait=new)
        if type(inst).__name__ in (
            "InstMatmult", "InstTensorCopy", "InstTensorTensor",
            "InstTensorScalarPtr", "InstActivation", "InstMemset",
            "InstTensorReduce", "InstTensorTensorReduce",
        ):
            streams[eng].append(inst)

    for eng, insts in streams.items():
        overflow = []
        for inst in reversed(insts):
            si = inst.sync_info
            waits = list(si.on_wait) + overflow
            ded = {}
            for w in waits:
                if w.ant_name not in ded or ded[w.ant_name].wait_value < w.wait_value:
                    ded[w.ant_name] = w
            waits = list(ded.values())
            if len(waits) <= 1:
                inst.sync_info = dataclasses.replace(si, on_wait=waits)
                overflow = []
            else:
                inst.sync_info = dataclasses.replace(si, on_wait=[waits[-1]])
                overflow = waits[:-1]
        if overflow:
            raise RuntimeError(f"{eng}: could not place {len(overflow)} waits")


def _build():
    import concourse.bass as bass
    import concourse.tile as tile
    from concourse import bacc, mybir

    f32 = mybir.dt.float32
    bf16 = mybir.dt.bfloat16
    ALU = mybir.AluOpType
    ACTF = mybir.ActivationFunctionType

    nc = bacc.Bacc()

    p_binT = nc.declare_dram_parameter("binT", [C, NIJ], f32, isOutput=False)
    p_xw = nc.declare_dram_parameter("xw", [128, 2, N + H], f32, isOutput=False)
    p_x = nc.declare_dram_parameter("x", [N, H], f32, isOutput=False)
    p_wbin = nc.declare_dram_parameter("wbin", [C, H], f32, isOutput=False)
    p_bbin = nc.declare_dram_parameter("bbin", [128, 2], f32, isOutput=False)
    p_watt = nc.declare_dram_parameter("watt", [128, 2], f32, isOutput=False)
    p_batt = nc.declare_dram_parameter("battc", [128, 1], f32, isOutput=False)
    p_eye = nc.declare_dram_parameter("eye", [128, 128], f32, isOutput=False)
    p_e4 = nc.declare_dram_parameter("e4", [IB, IB * L], f32, isOutput=False)
    p_out = nc.declare_dram_parameter("out", [N, H], f32, isOutput=True)

    with tile.TileContext(nc) as tc:
        with (
            tc.tile_pool(name="const", bufs=1) as cpool,
            tc.tile_pool(name="binchunk", bufs=3) as bcpool,
            tc.tile_pool(name="work", bufs=3) as wpool,
            tc.tile_pool(name="uwork", bufs=4) as upool,
            tc.tile_pool(name="srow", bufs=4) as srpool,
            tc.tile_pool(name="fin", bufs=1) as fpool,
            tc.tile_pool(name="pbin", bufs=6, space=bass.MemorySpace.PSUM) as pbpool,
            tc.tile_pool(name="pscore", bufs=2, space=bass.MemorySpace.PSUM) as pspool,
            tc.tile_pool(name="dram", bufs=1, space=bass.MemorySpace.DRAM) as dpool,
        ):
            # ---------------- constants / prep ----------------
            binT_bf = cpool.tile([C, NIJ], bf16, tag="binTbf")
            CH = 2048

            bb_sb = cpool.tile([128, 2], f32, tag="bb")
            nc.sync.dma_start(bb_sb[:, :], p_bbin[:, :])

            wa_sb = cpool.tile([128, 2], f32, tag="wa")
            nc.sync.dma_start(wa_sb[:, :], p_watt[:, :])

            xw_sb = cpool.tile([128, 2, N + H], f32, tag="xw")  # [p, h-tile, j | k]
            nc.sync.dma_start(xw_sb[:, 0, :], p_xw[:, 0, :])
            nc.sync.dma_start(xw_sb[:, 1, :], p_xw[:, 1, :])

            wbin_f = cpool.tile([C, H], f32, tag="wbinf")
            nc.sync.dma_start(wbin_f[:, :], p_wbin[:, :])
            wbin_bf = cpool.tile([C, H], bf16, tag="wbinbf")
            nc.scalar.copy(wbin_bf[:, :], wbin_f[:, :])
            # W4: per (kt, m) a [128,4] stationary with watt[kt] in col m, zeros else
            w4 = cpool.tile([128, 2, 2 * IB, 2 * IB], bf16, tag="w4")
            nc.vector.memset(w4[:, :, :, :], 0.0)
            for kt in range(2):
                for m in range(2 * IB):
                    nc.vector.tensor_copy(w4[:, kt, m, m : m + 1], wa_sb[:, kt : kt + 1])

            bc_sb = cpool.tile([128, 1], f32, tag="battc")
            nc.sync.dma_start(bc_sb[:, :], p_batt[:, :])
            sigwarm = cpool.tile([1, 1], bf16, tag="sigwarm")
            nc.scalar.activation(sigwarm[:, :], bc_sb[0:1, 0:1], ACTF.Sigmoid, bias=0.0, scale=1.0)

            x_f0 = cpool.tile([128, H], f32, tag="xf0")
            x_f1 = cpool.tile([64, H], f32, tag="xf1")
            nc.sync.dma_start(x_f0[:, :], p_x[0:128, :])
            nc.sync.dma_start(x_f1[:, :], p_x[128:192, :])
            x_bf0 = cpool.tile([128, H], bf16, tag="xbf0")
            x_bf1 = cpool.tile([64, H], bf16, tag="xbf1")
            nc.scalar.copy(x_bf0[:, :], x_f0[:, :])
            nc.scalar.copy(x_bf1[:, :], x_f1[:, :])

            # Y^T[k,j] = sum_h Wap[h,k] * XT[h,j]   (per k-tile)
            yt_f = cpool.tile([128, 2 * N], f32, tag="ytf")  # fp32, per-i bias source
            yt_b = cpool.tile([128, 2 * N], bf16, tag="ytb")  # bf16, region-B stream source
            base4 = cpool.tile([128, 2, IB, L], bf16, tag="base4")  # (Y^T + b_bin)[:, :128] x4
            for kt in range(2):
                psy = pspool.tile([128, H], f32, tag="score")
                for ht in range(2):
                    nc.tensor.matmul(
                        psy[:, 0:N],
                        xw_sb[:, ht, N + kt * 128 : N + (kt + 1) * 128],
                        xw_sb[:, ht, 0:N],
                        start=(ht == 0),
                        stop=(ht == 1),
                    )
                nc.vector.tensor_copy(yt_f[:, kt * N : (kt + 1) * N], psy[:, 0:N])
                nc.scalar.copy(yt_b[:, kt * N : (kt + 1) * N], psy[:, 0:N])
                nc.vector.tensor_scalar(
                    base4[:, kt, 0, :], psy[:, 0:L], bb_sb[:, kt : kt + 1], None, ALU.add
                )
                for g in range(1, IB):
                    nc.vector.tensor_copy(base4[:, kt, g, :], base4[:, kt, 0, :])

            eye_f = cpool.tile([128, 128], f32, tag="eyef")
            nc.sync.dma_start(eye_f[:, :], p_eye[:, :])
            eye_b = cpool.tile([128, 128], bf16, tag="eyeb")
            nc.scalar.copy(eye_b[:, :], eye_f[:, :])
            e4_f = cpool.tile([IB, IB * L], f32, tag="e4f")
            nc.sync.dma_start(e4_f[:, :], p_e4[:, :])
            e4_b = cpool.tile([IB, IB, L], bf16, tag="e4b")
            nc.scalar.copy(e4_b[:, :, :], e4_f[:, :])

            # Y in natural layout (rows i<128 on partitions) for the bias-fold matmul
            yn0 = cpool.tile([128, 2 * 128], bf16, tag="yn0")
            for kt in range(2):
                pst = pspool.tile([128, 128], bf16, tag="score")
                nc.tensor.transpose(pst[:, :], yt_b[:, kt * N : kt * N + 128], eye_b[:, :])
                nc.vector.tensor_copy(yn0[:, kt * 128 : (kt + 1) * 128], pst[:, :])
            # regroup: ynG[r, ib, :] = yn0[ib*4+r, :] so burst stationaries sit at partitions 0..3
            ynG = cpool.tile([IB, 32, 2 * 128], bf16, tag="ynG")
            for ib in range(32):
                nc.gpsimd.dma_start(ynG[:, ib, :], yn0[ib * IB : (ib + 1) * IB, :])

            # binT load + cast to bf16
            for ch in range(NIJ // CH):
                bchunk = bcpool.tile([C, CH], f32, tag="bchunk")
                nc.sync.dma_start(bchunk[:, :], p_binT[:, ch * CH : (ch + 1) * CH])
                nc.scalar.copy(binT_bf[:, ch * CH : (ch + 1) * CH], bchunk[:, :])

            scratch = dpool.tile([N, N], bf16, tag="scratch")

            # ---------------- main loops ----------------
            lo_state = {}
            hi_state = {}

            def score_rows(score_state, ib, i0, u, w):
                """Reduce IB i-rows; scores accumulate in psum stripes; flush every 2 bursts."""
                if ib % 2 == 0:
                    ps_t = pspool.tile([128, H], f32, tag="score")
                    score_state["ps"] = ps_t
                    score_state["first"] = True
                ps = score_state["ps"]
                sb = (ib % 2) * IB
                for kt in range(2):
                    for m in range(IB):
                        nc.tensor.matmul(
                            ps[0 : 2 * IB, 0:w],
                            w4[:, kt, sb + m, :],
                            u[:, kt, m, 0:w],
                            start=score_state["first"],
                            stop=(ib % 2 == 1 and kt == 1 and m == IB - 1),
                            skip_group_check=True,
                        )
                        score_state["first"] = False
                if ib % 2 == 1:
                    srow = srpool.tile([2 * IB, N], bf16, tag="srow")
                    nc.scalar.activation(
                        srow[:, 0:w], ps[0 : 2 * IB, 0:w], ACTF.Sigmoid,
                        bias=bc_sb[0 : 2 * IB, 0:1], scale=1.0,
                    )
                    nc.gpsimd.dma_start(scratch[i0 - IB : i0 + IB, 0:w], srow[:, 0:w])


            st0 = fpool.tile([128, N], bf16, tag="st0")
            st1 = fpool.tile([64, N], bf16, tag="st1")

            def hi_burst(ib):
                i0 = L + ib * IB
                u = upool.tile([128, 2, IB, N], bf16, tag="uhi")
                for kt in range(2):
                    for m in range(IB):
                        yi = yt_f[:, kt * N + i0 + m : kt * N + i0 + m + 1]
                        nc.vector.tensor_scalar(
                            u[:, kt, m, :],
                            yt_b[:, kt * N : (kt + 1) * N],
                            yi,
                            0.0,
                            ALU.add,
                            ALU.max,
                        )
                score_rows(hi_state, ib, i0, u, N)


            def lo_burst(ib):
                i0 = ib * IB
                pb = []
                for kt in range(2):
                    pbt = pbpool.tile([128, IB, L], f32, tag="pb")
                    for m in range(IB):
                        nc.tensor.matmul(
                            pbt[:, m, :],
                            wbin_bf[:, kt * 128 : (kt + 1) * 128],
                            binT_bf[:, (i0 + m) * L : (i0 + m + 1) * L],
                            start=(m == 0),
                            stop=False,
                            skip_group_check=True,
                        )
                    nc.tensor.matmul(
                        pbt[:, :, :],
                        ynG[:, ib, kt * 128 : (kt + 1) * 128],
                        e4_b[:, :, :],
                        start=False,
                        stop=True,
                        skip_group_check=True,
                    )
                    pb.append(pbt)
                z = wpool.tile([128, 2, IB, L], bf16, tag="z")
                zc = wpool.tile([128, IB, L], bf16, tag="zc")
                nc.scalar.copy(zc[:, :, :], pb[1][:, :, :])
                nc.vector.tensor_add(z[:, 1, :, :], zc[:, :, :], base4[:, 1, :, :])
                if ib >= 29:
                    zc2 = wpool.tile([128, IB, L], bf16, tag="zc2")
                    nc.scalar.copy(zc2[:, :, :], pb[0][:, :, :])
                    nc.vector.tensor_add(z[:, 0, :, :], zc2[:, :, :], base4[:, 0, :, :])
                else:
                    nc.vector.tensor_add(z[:, 0, :, :], pb[0][:, :, :], base4[:, 0, :, :])
                u = upool.tile([128, 2, IB, L], bf16, tag="u")
                nc.vector.tensor_scalar(
                    u[:, :, :, :], z[:, :, :, :], 0.0, None, ALU.max
                )
                score_rows(lo_state, ib, i0, u, L)



            for _h in range(7):
                hi_burst(_h)
            hi_next = 7
            for ck in range(8):
                for k in range(4):
                    lo_burst(ck * 4 + k)
                    if k % 2 == 1 and hi_next < 16:
                        hi_burst(hi_next)
                        hi_next += 1

            # hi rows complete: fill every score block that depends on them
            nc.sync.dma_start_transpose(st0[:, L:N], scratch[L:N, 0:L])
            nc.sync.dma_start(st1[:, 0:L], scratch[L:N, 0:L])
            nc.sync.dma_start_transpose(st1[:, L:N], scratch[L:N, L:N])

            # ---------------- finale: last transpose, sigmoid, S @ X ----------------
            nc.sync.dma_start_transpose(st0[:, 0:L], scratch[0:L, 0:L])

            for it, (lo, sz) in enumerate(((0, 128), (128, 64))):
                po = pspool.tile([128, H], f32, tag="score")
                nc.tensor.matmul(po[0:sz, :], st0[:, lo : lo + sz], x_bf0[:, :], start=True, stop=False)
                nc.tensor.matmul(po[0:sz, :], st1[:, lo : lo + sz], x_bf1[:, :], start=False, stop=True)
                ob = fpool.tile([sz, H], f32, tag=f"ob{it}")
                nc.vector.tensor_copy(ob[:, :], po[0:sz, :])
                nc.sync.dma_start(p_out[lo : lo + sz, :], ob[:, :])

    nc.compile()
    return nc


def _e4_const():
    e = np.zeros((IB, IB, L), np.float32)
    for m in range(IB):
        e[m, m, :] = 1.0
    return np.ascontiguousarray(e.reshape(IB, IB * L))


def _prep_inputs(local_feats, binary_feats, W_apair, W_bin, b_bin, w_att, b_att):
    lf = np.asarray(local_feats, np.float32)
    bf = np.asarray(binary_feats, np.float32)
    wap = np.ascontiguousarray(np.asarray(W_apair, np.float32))
    wbin = np.ascontiguousarray(np.asarray(W_bin, np.float32))
    bb = np.ascontiguousarray(np.asarray(b_bin, np.float32).reshape(H, 1))
    wa = np.ascontiguousarray(np.asarray(w_att, np.float32).reshape(H, 1))
    battc = np.full((128, 1), np.float32(np.asarray(b_att).reshape(-1)[0]), np.float32)
    in_maps = []
    for b in range(B):
        in_maps.append(
            {
                "binT": np.ascontiguousarray(bf[b].reshape(NIJ, C).T),
                "xw": np.ascontiguousarray(np.concatenate([
                    lf[b].T.reshape(2, 128, N).transpose(1, 0, 2),
                    wap.reshape(2, 128, H).transpose(1, 0, 2)], axis=2)),
                "x": np.ascontiguousarray(lf[b]),
                "wbin": wbin,
                "bbin": np.ascontiguousarray(bb.reshape(2, 128).T),
                "watt": np.ascontiguousarray(wa.reshape(2, 128).T),
                "battc": battc,
                "eye": np.eye(128, dtype=np.float32),
                "e4": _e4_const(),
            }
        )
    return in_maps


def run_full(inputs, trace=False):
    from concourse.bass_utils import run_bass_kernel_spmd

    if "nc" not in _CACHE:
        _CACHE["nc"] = _build()
    nc = _CACHE["nc"]
    in_maps = _prep_inputs(
        inputs["local_feats"],
        inputs["binary_feats"],
        inputs["W_apair"],
        inputs["W_bin"],
        inputs["b_bin"],
        inputs["w_att"],
        inputs["b_att"],
    )
    res = run_bass_kernel_spmd(nc, in_maps, list(range(B)), trace=trace)
    out = np.stack([np.asarray(res.results[c]["out"], np.float32) for c in range(B)])
    return out, res


def kernel(**inputs):
    out, _ = run_full(inputs, trace=False)
    return out

